# revision 65
# baseline (speedup 1.0000x reference)
"""Trainium2 Bass kernel for a diagonal-A linear dynamical system (LDS).

    Bu = inputs @ B            [B, T, S]
    h_t = h_{t-1} * A + Bu_t   (scan over T, diagonal A)
    y_t = h_t @ C              [B, T, O]

Shapes: inputs [16, 4096, 256], A [256], B [256, 256], C [256, 256],
h0 [256]; all float32.

Sharding: data-parallel over batch across 8 NeuronCores (2 batches per
core); A/B/C/h0 replicated.

v5 design: mixed-R batch split to balance PE vs DVE.

The DVE TensorTensorScan runs ~2.4 ns/col and no other engine supports
it; the PE runs 1 col/cycle @2.4GHz. Pure R=1 (Bu, scan, y=h@C) costs
65536 PE cols + 16384 scan cols (PE 27us / DVE 39us -> DVE-bound);
pure R=2 pair-step costs 81920 PE cols + 8192 scan cols (PE 37us /
DVE 19.5us -> PE-bound). Splitting the two per-core batches one each
way balances at PE ~31us / DVE ~29us:

  batch slot 0 (R=1): Bu_t = u_t @ B; h = scan(A, Bu); y = h @ C
  batch slot 1 (R=2): scan odd steps only,
      h_{2u+1} = A^2 h_{2u-1} + v_u,  v_u = u_{2u} @ B' + u_{2u+1} @ B
      y_{2u+1} = h_{2u+1} @ C
      y_{2u}   = h_{2u-1} @ C' + u_{2u} @ BC
  with B' = B diag(A), C' = diag(A) C, BC = B @ C precomputed on host
  (B, C scaled by 16 so BC stays fp16-normal; ACT PSUM->SBUF copies
  multiply y by 1/256 to undo it).

Other v5 changes vs v4:
  - DRAM tiles laid out contiguous per partition (2KB+ runs -> fewer
    DMA descriptors).
  - y DMAs issue on the idle Pool/SWDGE rail, u on Sync HWDGE, W1/W2
    on Pool, S on ACT HWDGE (an early ACT-rail instruction also forces
    the 1.3us ACT_TABLE_LOAD preamble to run before the first y copy
    needs the engine) -> three parallel issue rails.
  - PE warmup matmuls on garbage data during the initial DMA wait so
    the p-state ramp (~2.5us at half clock) overlaps the wait for the
    first u tile (DMA completion receipt alone is ~2us).
  - fine-grained round schedule: round r = [prefetch r+1 tiles; v(r);
    bu(2r); scan0(2r); scan1(r); y1(r-1); bu(2r+1); scan0(2r+1);
    y0(2r-1); y0(2r)] -- orders each engine's in-order queue so PE
    runs gap-free and scans complete just before their y consumers.
  - PSUM budget (8 banks): ps_bu 3 + ps_v 2 + ps_y 3.
fp16 data path everywhere; PSUM accumulation and scan state stay fp32.

Measured (8-core axon trn2): ~54-55us vs 57.7us for the v4 baseline;
PE dense-packed (two <2us gaps at startup only). Remaining time is
fixed overhead: 6.5-10us NEFF preamble, ~4us first-tile DMA latency,
~3us final drain, plus a ~31us PE span balanced against ~29us DVE.
Note: device clock varies +-20% run to run (matmul 216 vs 259+ ns);
compare traces via slice durations, not wall time alone.
"""

import numpy as np

import concourse.bacc as bacc
import concourse.bass as bass
import concourse.mybir as mybir
import concourse.tile as tile
from concourse import bass_utils

BATCH, T, D = 16, 4096, 256
NCORES = 8
BLOC = BATCH // NCORES  # batches per core (slot 0: R=1, slot 1: R=2)
T0 = 512                # R=1 chunk timesteps
NJ0 = T // T0           # 8 chunks
T1 = 1024               # R=2 chunk timesteps
SC = T1 // 2            # R=2 scan cols / matmul width per chunk
NJ1 = T // T1           # 4 chunks
HL0 = T + 1             # R=1 state buffer (incl. h0 guard col)
HL1 = T // 2 + 1        # R=2 odd-state buffer (incl. h0 guard col)
WSCALE = 16.0           # host scales B, C by this; y copy undoes ^2
F32 = mybir.dt.float32
F16 = mybir.dt.float16

_CACHE: dict = {}


def _build_nc():
    nc = bacc.Bacc(trn_type="TRN2", target_bir_lowering=False)

    # u0[j, p, k, e] = inputs[b0, j*T0 + e, k*128 + p]
    u0d = nc.dram_tensor("u0", [NJ0, 128, 2, T0], F16, kind="ExternalInput")
    # u1[j, p, k, par, e] = inputs[b1, j*T1 + 2*e + par, k*128 + p]
    u1d = nc.dram_tensor("u1", [NJ1, 128, 2, 2, SC], F16, kind="ExternalInput")
    # W1[p, w, k, d]: w=0 Bp=B*diag(A), w=1 B   (i = k*128+p)
    W1d = nc.dram_tensor("W1", [128, 2, 2, D], F16, kind="ExternalInput")
    # W2[p, w, k, d]: w=0 C, w=1 Cp=diag(A)*C, w=2 BC=B@C
    W2d = nc.dram_tensor("W2", [128, 3, 2, D], F16, kind="ExternalInput")
    # S[p, c]: c=0,1 A^2 halves; c=2,3 A halves; c=4,5 h0 halves
    Sd = nc.dram_tensor("S", [128, 6], F32, kind="ExternalInput")
    # y0[j, p, m, e] = out[b0, j*T0 + e, m*128+p] (scaled in-kernel)
    y0d = nc.dram_tensor("y0", [NJ0, 128, 2, T0], F16, kind="ExternalOutput")
    # y1[j, p, m, par, e] = out[b1, j*T1 + 2*e + par, m*128+p]
    y1d = nc.dram_tensor("y1", [NJ1, 128, 2, 2, SC], F16, kind="ExternalOutput")

    mult = mybir.AluOpType.mult
    add = mybir.AluOpType.add
    inv = 1.0 / (WSCALE * WSCALE)

    with tile.TileContext(nc) as tc:
        with (
            tc.tile_pool(name="const", bufs=1) as const,
            tc.tile_pool(name="u0pool", bufs=4) as u0pool,
            tc.tile_pool(name="u1pool", bufs=3) as u1pool,
            tc.tile_pool(name="y0pool", bufs=2) as y0pool,
            tc.tile_pool(name="y1pool", bufs=2) as y1pool,
            tc.tile_pool(name="hpool", bufs=1) as hpool,
            tc.tile_pool(name="ps_bu", bufs=3, space="PSUM") as ps_bu,
            tc.tile_pool(name="ps_v", bufs=2, space="PSUM") as ps_v,
            tc.tile_pool(name="ps_y", bufs=3, space="PSUM") as ps_y,
        ):
            u0_t: dict = {}
            u1_t: dict = {}

            def dma_u0(j, rail=None):
                t = u0pool.tile([128, 2, T0], F16, tag="u0", name="u0_t")
                u0_t[j] = t
                (rail or nc.sync).dma_start(t, u0d[j])

            def dma_u1(j):
                t = u1pool.tile([128, 2, 2, SC], F16, tag="u1", name="u1_t")
                # stored as (par0 view, par1 view); chunk 0 overrides with
                # two separate half tiles below
                u1_t[j] = (t[:, :, 0], t[:, :, 1])
                nc.sync.dma_start(t, u1d[j])

            # PE warmup: garbage matmuls to start the p-state ramp while
            # the first u tiles are still in flight. gpsimd.memset because
            # the Pool engine is ready ~1.4us before Vector in the preamble
            # (and these must precede the const DMA issues on the Pool seq).
            wa = const.tile([128, 128], F16, name="wa")
            nc.gpsimd.memset(wa, 0.0)
            wb = const.tile([128, 512], F16, name="wb")
            nc.gpsimd.memset(wb, 0.0)
            # 10 warmups ~= the preamble-to-first-tile DMA latency (~4.3us
            # at ramp speed); they keep the PE p-state ramp going right up
            # to the first real matmul instead of letting it decay, so the
            # real work starts at (near-)full clock.
            for _ in range(10):
                pw = ps_y.tile([128, 512], F32, tag="y_ps", name="warm")
                nc.tensor.matmul(pw, wa, wb, start=True, stop=True)

            # --- DMA issue order: consts on the Pool/SWDGE rail (the ACT
            # rail is blocked ~1.3us by ACT_TABLE_LOAD in the preamble);
            # first u0 chunk (256KB) gates the first real matmul -> first
            # on the Sync rail. ---
            W1 = const.tile([128, 2, 2, D], F16, name="W1")
            nc.gpsimd.dma_start(W1, W1d[:])
            dma_u0(0)
            W2 = const.tile([128, 3, 2, D], F16, name="W2")
            nc.gpsimd.dma_start(W2, W2d[:])
            # S on the ACT rail: tiny, and having an instruction queued
            # there up front makes the ACT preamble (ACT_TABLE_LOAD) run
            # immediately instead of stalling the first y copies later.
            Sc = const.tile([128, 6], F32, name="Sc")
            nc.scalar.dma_start(Sc, Sd[:])
            # chunk 0 of u1 loads as two par-half transfers: the v matmuls
            # are par-major, so v(0) starts after just the 256KB par0 half
            u10p = []
            for p in range(2):
                t = u1pool.tile([128, 2, SC], F16, tag=f"u1p{p}", name="u1p")
                u10p.append(t)
                nc.sync.dma_start(t, u1d[0][:, :, p])
            u1_t[0] = (u10p[0], u10p[1])
            # NOTE: keep all u transfers on the ONE sync rail in priority
            # order -- rails fair-share the 16 SDMA engines at packet
            # granularity, so a second rail would steal bandwidth from the
            # critical first tile rather than pipelining behind it.
            dma_u0(1)

            Bp_sb, B_sb = W1[:, 0], W1[:, 1]          # [128, 2, D]
            C_sb, Cp_sb, BC_sb = W2[:, 0], W2[:, 1], W2[:, 2]
            A2_col, A1_col, h0c = Sc[:, 0:2], Sc[:, 2:4], Sc[:, 4:6]

            ones = const.tile([128, SC], F32, name="ones")
            nc.vector.memset(ones, 1.0)
            A2_bc = const.tile([128, 2, SC], F32, name="A2_bc")
            A1_bc = const.tile([128, 2, SC], F32, name="A1_bc")
            for m in range(2):
                nc.scalar.mul(A2_bc[:, m], ones, mul=A2_col[:, m : m + 1])
                nc.scalar.mul(A1_bc[:, m], ones, mul=A1_col[:, m : m + 1])

            # state buffers; col 0 is the h0 guard
            hT0 = hpool.tile([128, 2, HL0], F16, name="hT0")
            hT1 = hpool.tile([128, 2, HL1], F16, name="hT1")
            for m in range(2):
                nc.scalar.copy(hT0[:, m, 0:1], h0c[:, m : m + 1])
                nc.scalar.copy(hT1[:, m, 0:1], h0c[:, m : m + 1])

            # ---------------- R=1 path (batch slot 0) ----------------
            bu_ps: dict = {}

            def emit_bu0(j):
                ut = u0_t[j]
                for m in range(2):
                    ms = slice(m * 128, (m + 1) * 128)
                    bu = ps_bu.tile([128, T0], F32, tag="bu", name="bu")
                    bu_ps[(j, m)] = bu
                    for k in range(2):
                        nc.tensor.matmul(
                            bu, B_sb[:, k, ms], ut[:, k],
                            start=(k == 0), stop=(k == 1),
                        )

            def emit_scan0(j):
                for m in range(2):
                    init = (
                        h0c[:, m : m + 1]
                        if j == 0
                        else hT0[:, m, j * T0 : j * T0 + 1]
                    )
                    nc.vector.tensor_tensor_scan(
                        hT0[:, m, 1 + j * T0 : 1 + (j + 1) * T0],
                        A1_bc[:, m],
                        bu_ps.pop((j, m)),
                        init,
                        op0=mult,
                        op1=add,
                    )

            def emit_y0(j):
                last = j >= NJ0 - 2
                ysb = y0pool.tile([128, 2, T0], F16, tag="y0_sb", name="y0_sb")
                for m in range(2):
                    ms = slice(m * 128, (m + 1) * 128)
                    yp = ps_y.tile([128, T0], F32, tag="y_ps", name="y0_ps")
                    for k in range(2):
                        nc.tensor.matmul(
                            yp, C_sb[:, k, ms],
                            hT0[:, k, 1 + j * T0 : 1 + (j + 1) * T0],
                            start=(k == 0), stop=(k == 1),
                        )
                    if last and m == 1:
                        # final two chunks: m1 copy on the idle-by-now DVE
                        # so the trailing copies drain in parallel
                        nc.vector.tensor_scalar_mul(ysb[:, m], yp, inv)
                    else:
                        nc.scalar.mul(ysb[:, m], yp, mul=inv)
                rail = nc.sync if j == NJ0 - 1 else nc.gpsimd
                rail.dma_start(y0d[j], ysb)

            # ---------------- R=2 path (batch slot 1) ----------------
            v_ps: dict = {}

            def emit_v1(j):
                ut0, ut1 = u1_t[j]
                for m in range(2):
                    v_ps[(j, m)] = ps_v.tile([128, SC], F32, tag="v", name="v")
                # par-major: all par0 (u_even @ B') matmuls first so chunk
                # 0 can start before its par1 half-tile lands
                for m in range(2):
                    ms = slice(m * 128, (m + 1) * 128)
                    for k in range(2):
                        nc.tensor.matmul(
                            v_ps[(j, m)], Bp_sb[:, k, ms], ut0[:, k],
                            start=(k == 0), stop=False,
                        )
                for m in range(2):
                    ms = slice(m * 128, (m + 1) * 128)
                    for k in range(2):
                        nc.tensor.matmul(
                            v_ps[(j, m)], B_sb[:, k, ms], ut1[:, k],
                            start=False, stop=(k == 1),
                        )

            def emit_scan1(j):
                for m in range(2):
                    init = (
                        h0c[:, m : m + 1]
                        if j == 0
                        else hT1[:, m, j * SC : j * SC + 1]
                    )
                    nc.vector.tensor_tensor_scan(
                        hT1[:, m, 1 + j * SC : 1 + (j + 1) * SC],
                        A2_bc[:, m],
                        v_ps.pop((j, m)),
                        init,
                        op0=mult,
                        op1=add,
                    )

            def emit_y1(j):
                ysb = y1pool.tile([128, 2, 2, SC], F16, tag="y1_sb", name="y1_sb")
                for m in range(2):
                    ms = slice(m * 128, (m + 1) * 128)
                    yod = ps_y.tile([128, SC], F32, tag="y_ps", name="yod")
                    yev = ps_y.tile([128, SC], F32, tag="y_ps", name="yev")
                    for k in range(2):
                        nc.tensor.matmul(
                            yod, C_sb[:, k, ms],
                            hT1[:, k, 1 + j * SC : 1 + (j + 1) * SC],
                            start=(k == 0), stop=(k == 1),
                        )
                    for k in range(2):
                        nc.tensor.matmul(
                            yev, Cp_sb[:, k, ms],
                            hT1[:, k, j * SC : (j + 1) * SC],
                            start=(k == 0), stop=False,
                        )
                    for k in range(2):
                        nc.tensor.matmul(
                            yev, BC_sb[:, k, ms],
                            u1_t[j][0][:, k],
                            start=False, stop=(k == 1),
                        )
                    nc.scalar.mul(ysb[:, m, 1, :], yod, mul=inv)
                    nc.scalar.mul(ysb[:, m, 0, :], yev, mul=inv)
                    if j == NJ1 - 1:
                        # last chunk: ship each m-half as its copies land,
                        # on the sync rail (done with u by now; parallel
                        # issue vs the Pool rail, lower HWDGE latency)
                        nc.sync.dma_start(y1d[j][:, m], ysb[:, m])
                if j < NJ1 - 1:
                    nc.gpsimd.dma_start(y1d[j], ysb)

            # ---------------- schedule: 4 rounds ----------------
            # round r covers b1 chunk r + b0 chunks 2r,2r+1. y1 lags one
            # round, y0 lags one chunk; scans are emitted in consumer-
            # urgency order (scan0(2r) unblocks both bu(2r+1)'s PSUM tiles
            # and y0(2r) later this round, so it goes before scan1(r)).
            # Round 0 interleaves v(0) between the b0 chunks: bu(1) then
            # neither waits on the ps_bu pool (scan0(0) drains during v(0))
            # nor on u0[1] (third in the sync queue, lands during v(0)).
            dma_u1(1)
            dma_u0(2)
            dma_u0(3)
            emit_bu0(0)
            emit_scan0(0)
            emit_v1(0)
            emit_scan1(0)
            # y0(0) before bu(1): its scan is long done, so it gives the
            # PE work while bu(1)'s u0[1] tile (4th transfer) lands
            emit_y0(0)
            emit_bu0(1)
            emit_scan0(1)
            for r in range(1, NJ1 - 1):
                dma_u1(r + 1)
                dma_u0(2 * r + 2)
                dma_u0(2 * r + 3)
                emit_v1(r)
                emit_bu0(2 * r)
                emit_scan0(2 * r)
                emit_scan1(r)
                emit_y1(r - 1)
                emit_bu0(2 * r + 1)
                emit_scan0(2 * r + 1)
                emit_y0(2 * r - 1)
                emit_y0(2 * r)
            # last round reordered for the shortest tail: scan1(3) first so
            # y1(3) (12 matmuls + 4 copies + 2 transfers) finishes inside
            # the round; only the cheap y0(7) trails the final scan.
            r = NJ1 - 1
            emit_v1(r)
            emit_bu0(2 * r)
            emit_scan1(r)
            emit_scan0(2 * r)
            emit_y1(r - 1)
            emit_bu0(2 * r + 1)
            emit_scan0(2 * r + 1)
            emit_y0(2 * r - 1)
            emit_y1(r)
            emit_y0(2 * r)
            emit_y0(2 * r + 1)

    nc.compile()
    return nc


def _get_nc():
    if "nc" not in _CACHE:
        _CACHE["nc"] = _build_nc()
    return _CACHE["nc"]


def make_in_maps(inputs, A, B, C, h0):
    u = np.asarray(inputs, dtype=np.float32).astype(np.float16)

    Af = np.asarray(A, np.float32)
    Bf = np.asarray(B, np.float32) * WSCALE
    Cf = np.asarray(C, np.float32) * WSCALE
    Bp = Bf * Af[None, :]          # B * diag(A)
    Cp = Cf * Af[:, None]          # diag(A) * C
    BC = Bf @ Cf                   # (16B) @ (16C) = 256 * B@C

    def wsplit(M):  # [256, 256] -> [128, 2, 256] (p, k, d)
        return M.reshape(2, 128, D).transpose(1, 0, 2)

    W1 = np.ascontiguousarray(
        np.stack([wsplit(Bp), wsplit(Bf)], axis=1)
    ).astype(np.float16)
    W2 = np.ascontiguousarray(
        np.stack([wsplit(Cf), wsplit(Cp), wsplit(BC)], axis=1)
    ).astype(np.float16)
    A2 = (Af * Af).reshape(2, 128).T
    A1 = Af.reshape(2, 128).T
    h02 = (np.asarray(h0, np.float32) * WSCALE).reshape(2, 128).T
    S = np.ascontiguousarray(
        np.concatenate([A2, A1, h02], axis=1), dtype=np.float32
    )
    core_consts = {"W1": W1, "W2": W2, "S": S}

    in_maps = []
    for c in range(NCORES):
        b0, b1 = 2 * c, 2 * c + 1
        # u0[j, p, k, e] = u[b0, j*T0+e, k*128+p]
        u0 = np.ascontiguousarray(
            u[b0].reshape(NJ0, T0, 2, 128).transpose(0, 3, 2, 1)
        )
        # u1[j, p, k, par, e] = u[b1, j*T1+2e+par, k*128+p]
        u1 = np.ascontiguousarray(
            u[b1].reshape(NJ1, SC, 2, 2, 128).transpose(0, 4, 3, 1, 2)
            .transpose(0, 1, 2, 4, 3)
        )
        in_maps.append({"u0": u0, "u1": u1, **core_consts})
    return in_maps


def kernel(inputs, A, B, C, h0, _trace=False):
    nc = _get_nc()
    in_maps = make_in_maps(inputs, A, B, C, h0)
    res = bass_utils.run_bass_kernel_spmd(
        nc, in_maps, core_ids=list(range(NCORES)), trace=_trace
    )
    outs = np.empty((BATCH, T, D), np.float32)
    for c, r in enumerate(res.results):
        # y0[j, p, m, e] -> [j, e, m, p] -> [T, D]
        outs[2 * c] = (
            r["y0"].astype(np.float32).transpose(0, 3, 2, 1).reshape(T, D)
        )
        # y1[j, p, m, par, e] -> [j, e, par, m, p] -> [T, D]
        outs[2 * c + 1] = (
            r["y1"].astype(np.float32).transpose(0, 4, 3, 2, 1).reshape(T, D)
        )
    if _trace:
        _CACHE["last_result"] = res
    return outs
